# revision 14
# baseline (speedup 1.0000x reference)
"""GAT layer (N=50000, E=1.6M, D=128, H=4) on 8 trn2 NeuronCores.

Node partition: each core owns 49 blocks of 128 src nodes plus every
edge leaving them. Per block, ONE batched dma_gather (4 rotating SWDGE
queues) fetches 512B rows [feat_bf16|1|snbr|sself|pad] for the edges'
dst nodes (int16 indices biased by -32768 so the full 50176-row table
is addressable; edges dst-sorted in-block for HBM locality; >=1
trailing non-negative pad index per block). s_self per edge comes from
host-shipped transposed one-hot matrices via TensorE matmuls against
the block's sself vector. Per 128-edge column, DVE builds 4 ex-scaled
one-hot matrices with fused is_equal*mult tensor_scalar ops (bf16 4x
mode) and TensorE accumulates [G_h|denom_h] per head in PSUM (bf16
matmuls, 1 cyc/col). ACT only uses exp/tanh/copy/relu (one table set;
sigmoid = 0.5*tanh(x/2)+0.5, leaky = max(x, 0.2x) on DVE).
"""

import numpy as np
import ml_dtypes

import concourse.bass as bass
import concourse.bacc as bacc
import concourse.mybir as mybir
import concourse.tile as tile
from concourse import library_config
from concourse.bass_utils import run_bass_kernel_spmd

N = 50000
E = 1_600_000
D = 128
H = 4
LEAKY = 0.2
N_CORES = 8
P = 128
BC = 49
NB = N_CORES * BC  # 392
NPAD = NB * P  # 50176
FATW = 256          # bf16 elems per fat row (512 B)
BIAS = 32768        # int16 index bias
GSUP = 8
QN = 4              # SWDGE queues for gather rotation
F32 = mybir.dt.float32
BF16 = mybir.dt.bfloat16
I16 = mybir.dt.int16
BF = ml_dtypes.bfloat16

_cache = {}


def _host_prep(edge_index):
    idx = np.asarray(edge_index).reshape(-1, 2)
    src = idx[:, 0].astype(np.int64)
    dst = idx[:, 1].astype(np.int64)
    blk = src >> 7
    order = np.lexsort((dst, blk))  # group by src block, dst-sorted inside
    src_s = src[order]
    dst_s = dst[order]
    cnt = np.bincount(blk[order], minlength=NB)
    starts = np.concatenate([[0], np.cumsum(cnt)])
    cnt2 = cnt.reshape(N_CORES, BC)
    # +1 guarantees >=1 trailing pad slot (trailing negative idx is skipped
    # by the gather ucode; pads use biased idx 0 which is non-negative)
    T_b = np.maximum(1, -(-(cnt2.max(axis=0) + 1) // P)).astype(int)
    offs = np.concatenate([[0], np.cumsum(T_b)]).astype(int)
    CT = int(offs[-1])

    dgi = np.zeros((N_CORES, P, CT * 8), np.int16)
    dgi2 = np.zeros((N_CORES, P, CT * 8), np.int16)
    scol = np.full((N_CORES, P, CT), 999.0, np.float32)
    for c in range(N_CORES):
        for lb in range(BC):
            b = c * BC + lb
            n = int(cnt[b])
            t = int(T_b[lb])
            o = int(offs[lb])
            s0 = starts[b]
            dpad = np.full(t * P, BIAS, np.int64)
            dpad[:n] = dst_s[s0 : s0 + n]
            w = (dpad - BIAS).astype(np.int16).reshape(t * 8, 16).T
            dgi[c, :, o * 8 : (o + t) * 8] = np.tile(w, (8, 1))
            spadg = np.full(t * P, BIAS, np.int64)
            spadg[:n] = src_s[s0 : s0 + n]
            w2 = (spadg - BIAS).astype(np.int16).reshape(t * 8, 16).T
            dgi2[c, :, o * 8 : (o + t) * 8] = np.tile(w2, (8, 1))
            spad = np.full(t * P, 999.0, np.float32)
            sl = (src_s[s0 : s0 + n] - b * P).astype(np.float32)
            spad[:n] = sl
            scol[c, :, o : o + t] = spad.reshape(t, P).T
    return T_b, offs, CT, dgi, dgi2, scol


def _make_in_maps(inputs, prep):
    T_b, offs, CT, dgi, dgi2, scol = prep
    feat = np.asarray(inputs["features"], np.float32)
    sk = np.asarray(inputs["self_kernels"], np.float32)
    ak = np.asarray(inputs["attn_kernels"], np.float32)
    gw = np.asarray(inputs["gate_weight"], np.float32)
    gb = np.asarray(inputs["gate_bias"], np.float32)

    fat = np.zeros((NPAD, FATW), BF)
    fat[:N, 0:D] = feat.astype(BF)
    fat[:, D] = BF(1.0)
    import os
    if os.environ.get("DBG_HOST_SN"):
        ftb = np.zeros((NPAD, D), np.float32)
        ftb[:N] = feat.astype(BF).astype(np.float32)
        wnbr = (sk * ak[:, D:]).T.astype(BF).astype(np.float32)
        wself = (sk * ak[:, :D]).T.astype(BF).astype(np.float32)
        fat[:, D + 1 : D + 1 + H] = (ftb @ wnbr).astype(BF)
        fat[:, D + 1 + H : D + 1 + 2 * H] = (ftb @ wself).astype(BF)
    ft = np.zeros((P, NPAD), BF)
    ft[:, :N] = feat.T.astype(BF)
    featp = np.zeros((NPAD, D), np.float32)
    featp[:N] = feat
    iota = np.tile(np.arange(P, dtype=np.float64), (P, 1)).astype(BF)
    skT = sk.T.copy().astype(np.float32)            # [128, 4]
    akT1 = ak[:, :D].T.copy().astype(np.float32)    # [128, 4]
    akT2 = ak[:, D:].T.copy().astype(np.float32)    # [128, 4]
    skbt_h = np.tile(sk.reshape(1, H * D), (P, 1)).astype(np.float32)
    gb2 = gb.reshape(1, D)

    in_maps = []
    for c in range(N_CORES):
        in_maps.append({
            "fat": fat,
            "ft": ft,
            "ftown": np.ascontiguousarray(ft[:, c * BC * P : (c + 1) * BC * P]),
            "f32o": np.ascontiguousarray(featp[c * BC * P : (c + 1) * BC * P]),
            "dgi": dgi[c],
            "dgi2": dgi2[c],
            "scolt": scol[c],
            "iota_t": iota,
            "skT": skT,
            "akT1": akT1,
            "akT2": akT2,
            "skbt": skbt_h,
            "gw32": gw,
            "gb32": gb2,
        })
    return in_maps


def _build(T_b, offs, CT, repeat=1):
    nc = bacc.Bacc("TRN2", target_bir_lowering=False, debug=False,
                   num_devices=N_CORES, num_swdge_queues=QN)

    fat = nc.dram_tensor("fat", [NPAD, FATW], BF16, kind="ExternalInput").ap()
    ft = nc.dram_tensor("ft", [P, NPAD], BF16, kind="ExternalInput").ap()
    ftownT = nc.dram_tensor("ftown", [P, BC * P], BF16, kind="ExternalInput").ap()
    f32o = nc.dram_tensor("f32o", [BC * P, D], F32, kind="ExternalInput").ap()
    dgi = nc.dram_tensor("dgi", [P, CT * 8], I16, kind="ExternalInput").ap()
    dgi2 = nc.dram_tensor("dgi2", [P, CT * 8], I16, kind="ExternalInput").ap()
    scolt = nc.dram_tensor("scolt", [P, CT], F32, kind="ExternalInput").ap()
    iota_t = nc.dram_tensor("iota_t", [P, P], BF16, kind="ExternalInput").ap()
    skTt = nc.dram_tensor("skT", [P, H], F32, kind="ExternalInput").ap()
    akT1t = nc.dram_tensor("akT1", [P, H], F32, kind="ExternalInput").ap()
    akT2t = nc.dram_tensor("akT2", [P, H], F32, kind="ExternalInput").ap()
    skbt = nc.dram_tensor("skbt", [P, H * D], F32, kind="ExternalInput").ap()
    gw32t = nc.dram_tensor("gw32", [D, D], F32, kind="ExternalInput").ap()
    gb32t = nc.dram_tensor("gb32", [1, D], F32, kind="ExternalInput").ap()
    outp = nc.dram_tensor("outp", [BC * P, D], F32, kind="ExternalOutput").ap()
    import os
    DBG = bool(os.environ.get("DBG_DUMP"))
    GONLY = int(os.environ.get("GONLY", "0"))
    NHEAD = int(os.environ.get("NHEAD", "4"))
    SKIP_SS = int(os.environ.get("SKIP_SS", "0"))
    if DBG:
        dbg_fg = nc.dram_tensor("dbg_fg", [P, int(max(T_b)), FATW], BF16,
                                kind="ExternalOutput").ap()
        dbg_ex = nc.dram_tensor("dbg_ex", [P, GSUP, H], F32,
                                kind="ExternalOutput").ap()
        dbg_ss = nc.dram_tensor("dbg_ss", [P, GSUP, H], F32,
                                kind="ExternalOutput").ap()
        dbg_g = nc.dram_tensor("dbg_g", [P, 4 * (D + 1)], F32,
                               kind="ExternalOutput").ap()

    ACT = mybir.ActivationFunctionType
    ALU = mybir.AluOpType

    with tile.TileContext(nc) as tc:
        with (
            tc.tile_pool(name="const", bufs=1) as cp,
            tc.tile_pool(name="work", bufs=2) as wp,
            tc.tile_pool(name="psum", bufs=1, space="PSUM") as pp,
        ):
            nc.gpsimd.load_library(library_config.mlp)

            # ---------- constants ----------
            iota_b = cp.tile([P, P], BF16)
            nc.sync.dma_start(iota_b[:], iota_t[:])
            skT_s = cp.tile([P, H], F32)
            nc.sync.dma_start(skT_s[:], skTt[:])
            akT1_s = cp.tile([P, H], F32)
            nc.sync.dma_start(akT1_s[:], akT1t[:])
            akT2_s = cp.tile([P, H], F32)
            nc.sync.dma_start(akT2_s[:], akT2t[:])
            skb32 = cp.tile([P, H * D], F32, tag="skb32")
            nc.sync.dma_start(skb32[:], skbt[:])
            gw_s = cp.tile([D, D], F32)
            nc.sync.dma_start(gw_s[:], gw32t[:])
            gb_s = cp.tile([1, D], F32)
            nc.sync.dma_start(gb_s[:], gb32t[:])

            # wcat8 = [wnbrT | wselfT] bf16 [128, 8]
            wcat8 = cp.tile([P, 2 * H], BF16)
            wtmp = cp.tile([P, H], F32, tag="wtmp")
            nc.vector.tensor_mul(wtmp[:], skT_s[:], akT2_s[:])
            nc.vector.tensor_copy(wcat8[:, 0:H], wtmp[:])
            nc.vector.tensor_mul(wtmp[:], skT_s[:], akT1_s[:])
            nc.vector.tensor_copy(wcat8[:, H : 2 * H], wtmp[:])

            ones1 = cp.tile([1, P], BF16)
            nc.vector.memset(ones1[:], 1.0)
            gwb = cp.tile([D, D], BF16)
            nc.vector.tensor_copy(gwb[:], gw_s[:])
            gbb = cp.tile([1, D], BF16)
            nc.vector.tensor_copy(gbb[:], gb_s[:])
            # skball [128, 512]: row p = [sk_0 | sk_1 | sk_2 | sk_3]
            skball = cp.tile([P, H * D], BF16)
            nc.vector.tensor_copy(skball[:], skb32[:])

            def _phases():
                # ---- Phase A: snbr/sself for all 392 blocks -> fat cols ----
                for g in range(NB // GSUP):
                    ftc = wp.tile([P, GSUP * P], BF16, tag="ftc", bufs=3)
                    nc.sync.dma_start(
                        ftc[:], ft[:, g * GSUP * P : (g + 1) * GSUP * P])
                    ps8 = pp.tile([P, GSUP * 2 * H], F32, tag="ps8", bufs=1,
                                  space="PSUM")
                    for k in range(GSUP):
                        nc.tensor.matmul(
                            ps8[:, k * 2 * H : (k + 1) * 2 * H],
                            ftc[:, k * P : (k + 1) * P],
                            wcat8[:],
                            start=True, stop=True)
                    sn_sb = wp.tile([P, GSUP * 2 * H], BF16, tag="snsb", bufs=3)
                    nc.vector.tensor_copy(sn_sb[:], ps8[:])
                    for k in range(GSUP):
                        b = g * GSUP + k
                        nc.sync.dma_start(
                            fat[b * P : (b + 1) * P, D + 1 : D + 1 + 2 * H],
                            sn_sb[:, k * 2 * H : (k + 1) * 2 * H])

                # featT of own blocks (gate lhsT)
                fto = cp.tile([P, BC * P], BF16, tag="fto")
                nc.sync.dma_start(fto[:], ftownT[:])

                # ---- Phase B: per own block ----
                Tmax = int(max(T_b))
                for lb in range(BC):
                    T = int(T_b[lb])
                    off = int(offs[lb])
                    fgf = wp.tile([P, Tmax, FATW], BF16, tag="fg", bufs=3)
                    nc.gpsimd.dma_gather(
                        out_ap=fgf[:, 0:T, :],
                        in_ap=fat[BIAS:NPAD, :],
                        idxs_ap=dgi_s[:, off * 8 : (off + T) * 8],
                        num_idxs=T * P,
                        num_idxs_reg=T * P,
                        elem_size=FATW,
                        single_packet=False,
                        queue_num=(2 * lb) % QN,
                    )
                    if DBG and lb == 0:
                        nc.sync.dma_start(dbg_fg[:, 0:T, :], fgf[:, 0:T, :])
                    sgf = wp.tile([P, Tmax, P], BF16, tag="sg", bufs=3)
                    nc.gpsimd.dma_gather(
                        out_ap=sgf[:, 0:T, :],
                        in_ap=fat[BIAS:NPAD, P : 2 * P],
                        idxs_ap=dgi2_s[:, off * 8 : (off + T) * 8],
                        num_idxs=T * P,
                        num_idxs_reg=T * P,
                        elem_size=P,
                        elem_step=FATW,
                        single_packet=False,
                        queue_num=(2 * lb + 1) % QN,
                    )

                    # full-bank tiles; head pairs share a bank, so only the
                    # first head's t==0 matmul may issue start=True (it clears
                    # has_written for the whole bank)
                    if GONLY:
                        continue
                    gp01 = pp.tile([P, 512], F32, tag="G01", bufs=2,
                                   space="PSUM", name="g01")
                    gp23 = pp.tile([P, 512], F32, tag="G23", bufs=2,
                                   space="PSUM", name="g23")
                    gsl = [gp01[:, 0 : D + 1], gp01[:, D + 1 : 2 * (D + 1)],
                           gp23[:, 0 : D + 1], gp23[:, D + 1 : 2 * (D + 1)]]

                    ngrp = (T + GSUP - 1) // GSUP
                    for gi in range(ngrp):
                        t0 = gi * GSUP
                        t1 = min(T, t0 + GSUP)
                        g = t1 - t0
                        # eraw = sself + snbr ; leaky = max(x, 0.2x); exp
                        eraw = wp.tile([P, GSUP, H], F32, tag="eraw", bufs=3)
                        nc.vector.tensor_add(
                            eraw[:, 0:g, :],
                            sgf[:, t0:t1, H + 1 : 2 * H + 1],
                            fgf[:, t0:t1, D + 1 : D + 1 + H])
                        esc = wp.tile([P, GSUP, H], F32, tag="esc", bufs=3)
                        nc.vector.tensor_scalar(
                            esc[:, 0:g, :], eraw[:, 0:g, :], LEAKY, None,
                            ALU.mult)
                        elr = wp.tile([P, GSUP, H], F32, tag="elr", bufs=3)
                        nc.vector.tensor_tensor(
                            elr[:, 0:g, :], eraw[:, 0:g, :], esc[:, 0:g, :],
                            op=ALU.max)
                        ex = wp.tile([P, GSUP, H], F32, tag="ex", bufs=3)
                        nc.scalar.activation(
                            ex[:, 0:g, :], elr[:, 0:g, :], ACT.Exp)
                        if DBG and lb == 0 and gi == 0:
                            exc = wp.tile([P, GSUP, H], F32, tag="exc")
                            nc.vector.tensor_copy(exc[:, 0:g, :], ssp[:, 0:g, :])
                            nc.sync.dma_start(dbg_ss[:], exc[:])
                            nc.sync.dma_start(dbg_ex[:], ex[:])
                        for t in range(t0, t1):
                            for h in range(NHEAD):
                                ohx = wp.tile([P, P], BF16, tag="ohx",
                                              bufs=GSUP, name="ohx")
                                nc.vector.tensor_scalar(
                                    ohx[:], iota_b[:],
                                    scol_s[:, off + t : off + t + 1],
                                    ex[:, t - t0, h : h + 1],
                                    ALU.is_equal, ALU.mult)
                                nc.tensor.matmul(
                                    gsl[h], ohx[:], fgf[:, t, 0 : D + 1],
                                    start=(t == 0 and h % 2 == 0),
                                    stop=(t == T - 1),
                                    skip_group_check=True)

                    if DBG and lb == 0:
                        gcp = wp.tile([P, 4 * (D + 1)], F32, tag="gcp")
                        nc.vector.tensor_copy(gcp[:, 0 : 2 * (D + 1)], gp01[:, 0 : 2 * (D + 1)])
                        nc.vector.tensor_copy(
                            gcp[:, 2 * (D + 1) : 4 * (D + 1)],
                            gp23[:, 0 : 2 * (D + 1)])
                        nc.sync.dma_start(dbg_g[:], gcp[:])
                    # ---- epilogue ----
                    den = wp.tile([P, H], F32, tag="den")
                    for h in range(H):
                        nc.vector.tensor_copy(
                            den[:, h : h + 1], gsl[h][:, D : D + 1])
                    den2 = wp.tile([P, H], F32, tag="den2")
                    nc.vector.tensor_scalar(
                        den2[:], den[:], 1.0e-30, float(H), ALU.max, ALU.mult)
                    rec = wp.tile([P, H], F32, tag="rec")
                    nc.vector.reciprocal(rec[:], den2[:])
                    acc = wp.tile([P, D], F32, tag="acc")
                    th2 = wp.tile([P, D], F32, tag="th2")
                    for h in range(H):
                        th = wp.tile([P, D], BF16, tag="th", bufs=2)
                        nc.scalar.activation(
                            th[:], gsl[h][:, 0:D], ACT.Copy,
                            scale=rec[:, h : h + 1])
                        if h == 0:
                            nc.vector.tensor_mul(
                                acc[:], th[:], skball[:, 0:D])
                        else:
                            nc.vector.tensor_mul(
                                th2[:], th[:], skball[:, h * D : (h + 1) * D])
                            nc.vector.tensor_add(acc[:], acc[:], th2[:])
                    oagg = wp.tile([P, D], F32, tag="oagg")
                    nc.scalar.activation(oagg[:], acc[:], ACT.Relu)

                    # gate: sigmoid(feat@gw+gb) = 0.5*tanh(0.5*x)+0.5
                    g_ps = pp.tile([P, D], F32, tag="gps", bufs=1, space="PSUM")
                    nc.tensor.matmul(
                        g_ps[:], fto[:, lb * P : (lb + 1) * P], gwb[:],
                        start=True, stop=False, skip_group_check=True)
                    nc.tensor.matmul(
                        g_ps[:], ones1[:], gbb[:],
                        start=False, stop=True, skip_group_check=True)
                    tgh = wp.tile([P, D], F32, tag="tgh")
                    nc.scalar.activation(tgh[:], g_ps[:], ACT.Tanh, scale=0.5)
                    gate = wp.tile([P, D], F32, tag="gate")
                    nc.vector.tensor_scalar(
                        gate[:], tgh[:], 0.5, 0.5, ALU.mult, ALU.add)

                    fco = wp.tile([P, D], F32, tag="fco", bufs=2)
                    nc.sync.dma_start(fco[:], f32o[lb * P : (lb + 1) * P, :])
                    dif = wp.tile([P, D], F32, tag="dif")
                    nc.vector.tensor_sub(dif[:], oagg[:], fco[:])
                    gd = wp.tile([P, D], F32, tag="gd")
                    nc.vector.tensor_mul(gd[:], gate[:], dif[:])
                    res = wp.tile([P, D], F32, tag="res", bufs=2)
                    nc.vector.tensor_add(res[:], fco[:], gd[:])
                    nc.sync.dma_start(outp[lb * P : (lb + 1) * P, :], res[:])

            # index/scalar tables stay resident
            dgi_s = cp.tile([P, CT * 8], I16, tag="dgis")
            nc.sync.dma_start(dgi_s[:], dgi[:])
            dgi2_s = cp.tile([P, CT * 8], I16, tag="dgi2s")
            nc.sync.dma_start(dgi2_s[:], dgi2[:])
            scol_s = cp.tile([P, CT], F32, tag="scols")
            nc.sync.dma_start(scol_s[:], scolt[:])

            if repeat == 1:
                _phases()
            else:
                with tc.For_i(0, repeat, 1):
                    _phases()

    nc.compile()
    return nc


def kernel(edge_index, features, self_kernels, attn_kernels, gate_weight,
           gate_bias):
    prep = _host_prep(edge_index)
    T_b, offs, CT = prep[0], prep[1], prep[2]
    key = ("prog", CT, tuple(int(x) for x in T_b))
    if key not in _cache:
        _cache[key] = _build(T_b, offs, CT)
    nc = _cache[key]
    in_maps = _make_in_maps(
        dict(features=features, self_kernels=self_kernels,
             attn_kernels=attn_kernels, gate_weight=gate_weight,
             gate_bias=gate_bias), prep)
    res = run_bass_kernel_spmd(nc, in_maps, core_ids=list(range(N_CORES)))
    out = np.concatenate([res.results[c]["outp"] for c in range(N_CORES)],
                         axis=0)
    return out[:N].astype(np.float32)
